# revision 1
# baseline (speedup 1.0000x reference)
"""Trainium2 Bass kernel for 16-head causal MultiHeadAttention.

Problem: N=4, T=2048, D_M=1024, HEADS=16, D_K=64, fp32, causal + key pad mask.

Sharding (8 cores): core c handles batch n = c//2 and head group g = c%2
(8 heads each).  Each core computes its batch's projections restricted to its
8 heads, causal attention for those heads, and a partial output projection
(A_heads @ Wo_rows).  The host sums the two partials per batch.

Device dataflow (transpose-free):
  - Host passes q/k/v pre-transposed (xT: [D_M, T]) so the projection
    contraction dim (d) lands on SBUF partitions.
  - qhT/khT ([j, T], head dim on partitions) come straight from the
    projection matmuls; vh ([T, dv]) likewise.
  - Scores are computed transposed, S^T[tk, tq] = khT^T-slice @ qhT-slice,
    so softmax's exp is elementwise from PSUM and attn@V consumes exp(S^T)
    directly: A^T[dv, tq] = vh^T @ exp(S^T) with vh in natural layout.
  - A ones-column appended to vh makes row 64 of the attn@V accumulator the
    softmax denominator (sum_tk exp) for free.
  - A_norm^T is exactly the lhsT the output projection needs; out[tq, e]
    comes out in natural layout for the store.
All matmuls run as float32r (full PE rate, fp32 storage).
"""

import os
import sys

import numpy as np

for _p in ("/opt/trn_rl_repo",):
    if _p not in sys.path and os.path.isdir(_p):
        sys.path.insert(0, _p)

import concourse.bacc as bacc
import concourse.bass as bass
import concourse.mybir as mybir
import concourse.tile as tile
from concourse.bass_utils import run_bass_kernel_spmd

# Problem constants (hardcoded per harness contract).
D_M = 1024
HEADS = 16
D_K = 64
N_B = 4
T = 2048
N_CORES = 8
HPC = HEADS // 2          # heads per core = 8
J = HPC * D_K             # per-core projection width = 512
G = J // 128              # j-tiles per core = 4
DT = D_M // 128           # d (contraction) tiles = 8
CHUNK = 512               # tq chunk (free dim of most matmuls)
NCHUNK = T // CHUNK       # 4
NBLK = T // 128           # tk blocks = 16
F32 = mybir.dt.float32
F32R = mybir.dt.float32r
NEG = -1.0e30

_cached_nc = None


def r(ap):
    """float32r view (no-op for tiles already declared float32r)."""
    return ap if ap.dtype == F32R else ap.bitcast(F32R)


def build_nc(loop_n=None, phases=('kv', 'q', 'attn', 'out')):
    """loop_n: if set, wrap the compute body in a HW For_i loop (timing
    variant — K projection reuses the Q weights so weight slots load once;
    outputs are numerically wrong but the instruction stream is identical)."""
    import contextlib
    nc = bacc.Bacc(None)

    xq = nc.declare_dram_parameter("xq_t", [D_M, T], F32R, isOutput=False)
    xk = nc.declare_dram_parameter("xk_t", [D_M, T], F32R, isOutput=False)
    xv = nc.declare_dram_parameter("xv_t", [D_M, T], F32R, isOutput=False)
    wq = nc.declare_dram_parameter("wq", [D_M, J], F32R, isOutput=False)
    wk = nc.declare_dram_parameter("wk", [D_M, J], F32R, isOutput=False)
    wv = nc.declare_dram_parameter("wv", [D_M, J], F32R, isOutput=False)
    bq = nc.declare_dram_parameter("bq2", [128, G], F32, isOutput=False)
    bk = nc.declare_dram_parameter("bk2", [128, G], F32, isOutput=False)
    bvb = nc.declare_dram_parameter("bvb", [128, J], F32, isOutput=False)
    wo = nc.declare_dram_parameter("wo", [J, D_M], F32R, isOutput=False)
    bob = nc.declare_dram_parameter("bob", [128, D_M], F32, isOutput=False)
    padb = nc.declare_dram_parameter("padb", [128, NBLK], F32, isOutput=False)
    trim = nc.declare_dram_parameter("trimask", [128, 128], F32, isOutput=False)
    out = nc.declare_dram_parameter("out", [T, D_M], F32, isOutput=True)

    Ident = mybir.ActivationFunctionType.Identity
    Exp = mybir.ActivationFunctionType.Exp

    with tile.TileContext(nc) as tc:
        with (
            tc.tile_pool(name="consts", bufs=1) as cpool,
            tc.tile_pool(name="wproj", bufs=1) as wpool,
            tc.tile_pool(name="persist", bufs=1) as ppool,
            tc.tile_pool(name="xs", bufs=16) as xpool,
            tc.tile_pool(name="qht", bufs=6) as qpool,
            tc.tile_pool(name="ant", bufs=6) as apool,
            tc.tile_pool(name="exps", bufs=3) as epool,
            tc.tile_pool(name="rec", bufs=2) as rpool,
            tc.tile_pool(name="osb", bufs=3) as opool,
            tc.tile_pool(name="ps_x", bufs=3, space="PSUM") as psum_px,
            tc.tile_pool(name="ps_a", bufs=2, space="PSUM") as psum_ap,
        ):
            # ---- constants -------------------------------------------------
            trim_t = cpool.tile([128, 128], F32, name="trim_t", tag="trim")
            nc.sync.dma_start(out=trim_t[:, :], in_=trim[:, :])
            padb_t = cpool.tile([128, NBLK], F32, name="padb_t", tag="padb")
            nc.sync.dma_start(out=padb_t[:, :], in_=padb[:, :])
            bq_t = cpool.tile([128, G], F32, name="bq_t", tag="bq")
            nc.sync.dma_start(out=bq_t[:, :], in_=bq[:, :])
            bk_t = cpool.tile([128, G], F32, name="bk_t", tag="bk")
            nc.sync.dma_start(out=bk_t[:, :], in_=bk[:, :])
            bvb_t = cpool.tile([128, J], F32, name="bvb_t", tag="bvb")
            nc.sync.dma_start(out=bvb_t[:, :], in_=bvb[:, :])
            bob_t = cpool.tile([128, D_M], F32, name="bob_t", tag="bob")
            nc.sync.dma_start(out=bob_t[:, :], in_=bob[:, :])

            # Output-projection weights, resident: wo_t[g] = wo[128g:+128, :]
            wo_t = []
            for g in range(G):
                t_ = wpool.tile([128, D_M], F32R, name=f"wo_t{g}", tag=f"wo{g}")
                nc.sync.dma_start(out=t_[:, :], in_=wo[g * 128:(g + 1) * 128, :])
                wo_t.append(t_)

            # V-projection rhs weights, resident: wv_t[d] = wv[128d:+128, :]
            wv_t = []
            for d in range(DT):
                t_ = wpool.tile([128, J], F32R, name=f"wv_t{d}", tag=f"wv{d}")
                nc.sync.dma_start(out=t_[:, :], in_=wv[d * 128:(d + 1) * 128, :])
                wv_t.append(t_)

            # Q/K projection weights [128,128] tiles. wk first; wq reuses the
            # same slots after the K projection finishes (shared tags).
            def load_w(dram, label):
                ts_ = {}
                for d in range(DT):
                    for g in range(G):
                        t_ = wpool.tile(
                            [128, 128], F32R, name=f"{label}_{d}_{g}",
                            tag=f"wqk{d}_{g}", bufs=1,
                        )
                        nc.sync.dma_start(
                            out=t_[:, :],
                            in_=dram[d * 128:(d + 1) * 128, g * 128:(g + 1) * 128],
                        )
                        ts_[(d, g)] = t_
                return ts_

            wk_t = load_w(wk, "wk") if loop_n is None else None

            # Persistent activations.
            khT = []  # khT[g]: [128, T] — heads 2g (rows 0-63), 2g+1 (64-127)
            for g in range(G):
                t_ = ppool.tile([128, T], F32R, name=f"khT{g}", tag=f"khT{g}")
                khT.append(t_)
            vh = []   # vh[i]: [128, 520] — per head h: cols 65h..65h+63 = v, 65h+64 = 1
            for i in range(NBLK):
                t_ = ppool.tile([128, 520], F32R, name=f"vh{i}", tag=f"vh{i}")
                vh.append(t_)

            def load_x_chunk(xdram, c, label):
                tiles = []
                for d in range(DT):
                    t_ = xpool.tile([128, CHUNK], F32R, name=f"{label}{c}_{d}", tag="xs")
                    nc.sync.dma_start(
                        out=t_[:, :],
                        in_=xdram[d * 128:(d + 1) * 128, c * CHUNK:(c + 1) * CHUNK],
                    )
                    tiles.append(t_)
                return tiles

            def body(wk_t, wq_t, phases=phases):
                # ---- K projection -----------------------------------------
                for c in range(NCHUNK if 'kv' in phases else 0):
                    xk_c = load_x_chunk(xk, c, "xk")
                    for g in range(G):
                        ps = psum_px.tile([128, CHUNK], F32, name=f"psK{c}_{g}", tag="px", padded_shape=[128, 2 * CHUNK])
                        for d in range(DT):
                            nc.tensor.matmul(
                                ps[:, :], r(wk_t[(d, g)][:, :]), r(xk_c[d][:, :]),
                                start=(d == 0), stop=(d == DT - 1),
                            )
                        nc.vector.tensor_scalar_add(
                            khT[g][:, c * CHUNK:(c + 1) * CHUNK], ps[:, :],
                            bk_t[:, g:g + 1],
                        )

                # ---- V projection ---------------------------------------------
                for c in range(NCHUNK if 'kv' in phases else 0):
                    xv_c = load_x_chunk(xv, c, "xv")
                    for tl in range(4):
                        i = 4 * c + tl
                        ps = psum_px.tile([128, J], F32, name=f"psV{i}", tag="px", padded_shape=[128, 2 * CHUNK])
                        for d in range(DT):
                            nc.tensor.matmul(
                                ps[:, :],
                                r(xv_c[d][:, tl * 128:(tl + 1) * 128]),
                                r(wv_t[d][:, :]),
                                start=(d == 0), stop=(d == DT - 1),
                            )
                        # vh[i][:, 65h + (0..63)] = psum + bv ; vh[i][:, 65h+64] = 1
                        dst = vh[i][:, 0:J + HPC].rearrange("p (h e) -> p h e", e=65)
                        nc.vector.tensor_add(
                            dst[:, :, 0:64],
                            ps[:, :].rearrange("p (h e) -> p h e", e=64),
                            bvb_t[:, :].rearrange("p (h e) -> p h e", e=64),
                        )
                        nc.vector.memset(dst[:, :, 64:65].bitcast(F32), 1.0)

                if 'kv' not in phases and 'attn' in phases:
                    for g in range(G):
                        nc.vector.memset(khT[g][:, :].bitcast(F32), 0.0)
                    for i in range(NBLK):
                        nc.vector.memset(vh[i][:, :].bitcast(F32), 0.0)

                # ---- Q weights (reuse wk slots) --------------------------------
                if loop_n is None:
                    wq_t = load_w(wq, "wq")

                # ---- per-chunk: Q proj -> attention -> out proj ----------------
                for c in range(NCHUNK):
                    qht = []
                    if 'q' in phases:
                        xq_c = load_x_chunk(xq, c, "xq")
                    for g in range(G):
                        qt = qpool.tile([128, CHUNK], F32R, name=f"qht{c}_{g}", tag="qht")
                        if 'q' not in phases and 'attn' in phases:
                            nc.vector.memset(qt[:, :].bitcast(F32), 0.0)
                        if 'q' in phases:
                            ps = psum_px.tile([128, CHUNK], F32, name=f"psQ{c}_{g}", tag="px", padded_shape=[128, 2 * CHUNK])
                            for d in range(DT):
                                nc.tensor.matmul(
                                    ps[:, :], r(wq_t[(d, g)][:, :]), r(xq_c[d][:, :]),
                                    start=(d == 0), stop=(d == DT - 1),
                                )
                            nc.vector.tensor_scalar_add(
                                qt[:, :], ps[:, :], bq_t[:, g:g + 1],
                            )
                        qht.append(qt)

                    nb = 4 * c + 4  # causal: tk blocks 0..nb-1
                    ant = []
                    for g in range(G):
                        at = apool.tile([128, CHUNK], F32R, name=f"ant{c}_{g}", tag="ant")
                        ant.append(at)
                        if 'attn' not in phases and 'out' in phases:
                            nc.vector.memset(at[:, :].bitcast(F32), 0.0)
                        if 'attn' not in phases:
                            continue
                        ps_a = [psum_ap.tile([65, CHUNK], F32,
                                             name=f"psA{c}_{2 * g + hh}", tag="pa")
                                for hh in range(2)]
                        # 2-deep software pipeline: A(bk-2) is emitted after
                        # S(bk), so each A pair waits on an exp that had a
                        # full block of ACT time to finish.  Both heads of a
                        # block share one [128,1024] PSUM pair-tile (2 banks)
                        # and one [128,1024] exp tile -> a single ACT op per
                        # block pair.
                        pend = []  # [(cs, es_pair), ...] oldest first
                        def emit_a(bk_, first):
                            pcs, pes = pend.pop(0)
                            for hh in range(2):
                                nc.tensor.matmul(
                                    ps_a[hh][:, pcs:],
                                    r(vh[bk_][:, 65 * (2 * g + hh):65 * (2 * g + hh) + 65]),
                                    r(pes[:, 512 * hh + pcs:512 * hh + 512]),
                                    start=first, stop=(bk_ == nb - 1),
                                )
                        for bk in range(nb):
                            m = bk - 4 * c  # >=0 on the diagonal superblock
                            cs = min(128 * m, 256) if m >= 0 else 0  # compute start
                            ms = 128 * m if m >= 0 else 0            # causal start
                            ps_s = psum_px.tile([128, 2 * CHUNK], F32,
                                                name=f"psS{c}_{g}_{bk}", tag="px")
                            for hh in range(2):
                                nc.tensor.matmul(
                                    ps_s[:, 512 * hh + cs:512 * hh + 512],
                                    r(khT[g][hh * 64:(hh + 1) * 64, bk * 128:(bk + 1) * 128]),
                                    r(qht[g][hh * 64:(hh + 1) * 64, cs:]),
                                    start=True, stop=True,
                                )
                            if m >= 0:
                                # triangular additive mask on both heads' diag blocks
                                for hh in range(2):
                                    nc.vector.tensor_add(
                                        ps_s[:, 512 * hh + ms:512 * hh + ms + 128],
                                        ps_s[:, 512 * hh + ms:512 * hh + ms + 128],
                                        trim_t[:, :],
                                    )
                            if bk >= 2:
                                emit_a(bk - 2, first=(bk == 2))
                            es = epool.tile([128, 2 * CHUNK], F32R,
                                            name=f"es{c}_{g}_{bk}", tag="es")
                            if ms > 0:
                                # one strided ACT op covering [ms:512] of both halves
                                nc.scalar.activation(
                                    es[:, 0:2 * CHUNK].rearrange(
                                        "p (h e) -> p h e", h=2)[:, :, ms:],
                                    ps_s[:, 0:2 * CHUNK].rearrange(
                                        "p (h e) -> p h e", h=2)[:, :, ms:],
                                    Exp, bias=padb_t[:, bk:bk + 1], scale=0.125,
                                )
                                if ms > cs:
                                    for hh in range(2):
                                        nc.vector.memset(
                                            es[:, 512 * hh + cs:512 * hh + ms].bitcast(F32), 0.0)
                            else:
                                nc.scalar.activation(
                                    es[:, :], ps_s[:, :], Exp,
                                    bias=padb_t[:, bk:bk + 1], scale=0.125,
                                )
                            pend.append((cs, es))
                        emit_a(nb - 2, first=False)
                        emit_a(nb - 1, first=False)
                        for hh in range(2):
                            h = 2 * g + hh
                            # normalize: rows 0-63 = A^T numerator, row 64 = denom.
                            # NB: partition_broadcast reads partition 0 of the
                            # underlying tile regardless of the input AP's
                            # partition offset, so the reciprocal must land on
                            # partition 0 (cross-base DVE write is fine on HW).
                            rc = rpool.tile([128, CHUNK], F32, name=f"rc{c}_{h}", tag="rc")
                            nc.vector.reciprocal(rc[0:1, :], ps_a[hh][64:65, :])
                            rb = rpool.tile([128, CHUNK], F32, name=f"rb{c}_{h}", tag="rb")
                            nc.gpsimd.partition_broadcast(rb[0:64, :], rc[0:1, :])
                            nc.vector.tensor_mul(
                                at[hh * 64:(hh + 1) * 64, :], ps_a[hh][0:64, :], rb[0:64, :],
                            )

                    # out[tq, e] = sum_g ant[g][:, tq-tile].T @ wo_t[g] + bo
                    for mt in range(4 if 'out' in phases else 0):
                        row0 = (4 * c + mt) * 128
                        for e in range(2):
                            ps = psum_px.tile([128, CHUNK], F32,
                                              name=f"psO{c}_{mt}_{e}", tag="px",
                                              padded_shape=[128, 2 * CHUNK])
                            for g in range(G):
                                nc.tensor.matmul(
                                    ps[:, :],
                                    r(ant[g][:, mt * 128:(mt + 1) * 128]),
                                    r(wo_t[g][:, e * CHUNK:(e + 1) * CHUNK]),
                                    start=(g == 0), stop=(g == G - 1),
                                )
                            ob = opool.tile([128, CHUNK], F32, name=f"ob{c}_{mt}_{e}", tag="ob")
                            nc.vector.tensor_add(ob[:, :], ps[:, :],
                                                 bob_t[:, e * CHUNK:(e + 1) * CHUNK])
                            nc.sync.dma_start(
                                out=out[row0:row0 + 128, e * CHUNK:(e + 1) * CHUNK],
                                in_=ob[:, :],
                            )

            if loop_n is not None:
                wq_t = load_w(wq, "wq")
                with tc.For_i(0, loop_n, 1):
                    body(wq_t, wq_t)
            else:
                body(wk_t, None)

    nc.finalize()
    return nc


def get_nc():
    global _cached_nc
    if _cached_nc is None:
        _cached_nc = build_nc()
    return _cached_nc


def make_in_maps(q, k, v, pad_mask, Wq, bq, Wk, bk, Wv, bv, Wo, bo):
    """Host-side sharding: core c -> batch c//2, head-group c%2."""
    f = np.float32
    tri = np.where(
        np.arange(128)[None, :] >= np.arange(128)[:, None], 0.0, NEG
    ).astype(f)  # [tk, tq]: allow tq >= tk
    in_maps = []
    xT = {}
    for n in range(N_B):
        xT[n] = (
            np.ascontiguousarray(np.asarray(q[n], f).T),
            np.ascontiguousarray(np.asarray(k[n], f).T),
            np.ascontiguousarray(np.asarray(v[n], f).T),
        )
    for c in range(N_CORES):
        n, grp = divmod(c, 2)
        js = slice(grp * J, (grp + 1) * J)
        pb = np.where(np.asarray(pad_mask[n]) == 0, NEG, 0.0).astype(f)
        in_maps.append({
            "xq_t": xT[n][0], "xk_t": xT[n][1], "xv_t": xT[n][2],
            "wq": np.ascontiguousarray(np.asarray(Wq, f)[:, js]),
            "wk": np.ascontiguousarray(np.asarray(Wk, f)[:, js]),
            "wv": np.ascontiguousarray(np.asarray(Wv, f)[:, js]),
            "bq2": np.ascontiguousarray(np.asarray(bq, f)[js].reshape(G, 128).T),
            "bk2": np.ascontiguousarray(np.asarray(bk, f)[js].reshape(G, 128).T),
            "bvb": np.broadcast_to(np.asarray(bv, f)[js], (128, J)).copy(),
            "wo": np.ascontiguousarray(np.asarray(Wo, f)[js, :]),
            "bob": (np.broadcast_to(np.asarray(bo, f), (128, D_M)).copy()
                    if grp == 0 else np.zeros((128, D_M), f)),
            "padb": np.ascontiguousarray(pb.reshape(NBLK, 128).T),
            "trimask": tri,
        })
    return in_maps


def kernel(**inputs) -> np.ndarray:
    nc = get_nc()
    in_maps = make_in_maps(**inputs)
    res = run_bass_kernel_spmd(nc, in_maps, list(range(N_CORES))).results
    out = np.empty((N_B, T, D_M), np.float32)
    for n in range(N_B):
        out[n] = res[2 * n]["out"] + res[2 * n + 1]["out"]
    return out



# revision 2
# speedup vs baseline: 1.1634x; 1.1634x over previous
"""Trainium2 Bass kernel for 16-head causal MultiHeadAttention (v2, bf16).

Problem: N=4, T=2048, D_M=1024, HEADS=16, D_K=64, fp32 in/out, causal +
key pad mask.

Sharding (8 cores): core c handles batch n = c//2 and head group g = c%2
(8 heads each).  Each core computes its batch's projections restricted to
its 8 heads, causal attention for those heads, and a partial output
projection (A_heads @ Wo_rows).  The host sums the two partials per batch.

v2 changes vs the fp32r baseline:
  - All SBUF tensors and matmuls are bf16 (fp32 PSUM accumulation), which
    halves DMA traffic (the run was DMA-saturated for its first ~100us)
    and SBUF footprint.  Final output stays fp32.
  - Wq gets its own SBUF slots (no reuse of the Wk slots), removing the
    serialized Wq load between the K projection and the Q projections.
  - The causal mask moves from DVE additive -1e30 on PSUM to a gpsimd
    affine_select (upper-triangle zero of exp(S^T) in SBUF), freeing DVE.
  - Per chunk, attention runs as ONE software-pipelined stream over all
    (head-group, key-block) pairs: A matmuls lag scores by LAG blocks
    ACROSS group boundaries, and the PE-only "filler" chains (output
    projection of the previous chunk, Q projections of later groups) are
    interleaved into the stream so the PE never drains while the ACT
    engine catches up on exp.
  - bf16 matmuls have no >=256-column constraint (fp32r did), so causal
    trimming of score/AV matmul columns is exact at 128 granularity.

Device dataflow (transpose-free), unchanged from the baseline:
  - Host passes q/k/v pre-transposed (xT: [D_M, T]) so the projection
    contraction dim lands on SBUF partitions.
  - qhT/khT ([j, T], head dim on partitions) come straight from the
    projection matmuls; vh ([T, dv]) likewise.
  - Scores are computed transposed, S^T[tk, tq] = khT-slice^T @ qhT-slice,
    so softmax's exp is elementwise from PSUM and attn@V consumes exp(S^T)
    directly: A^T[dv, tq] = vh^T @ exp(S^T).
  - A ones-column appended to vh makes row 64 of the attn@V accumulator
    the softmax denominator for free.
  - A_norm^T is exactly the lhsT the output projection needs.
"""

import os
import sys

import numpy as np

for _p in ("/opt/trn_rl_repo",):
    if _p not in sys.path and os.path.isdir(_p):
        sys.path.insert(0, _p)

import ml_dtypes

import concourse.bacc as bacc
import concourse.bass as bass
import concourse.mybir as mybir
import concourse.tile as tile
from concourse.bass_utils import run_bass_kernel_spmd

# Problem constants (hardcoded per harness contract).
D_M = 1024
HEADS = 16
D_K = 64
N_B = 4
T = 2048
N_CORES = 8
HPC = HEADS // 2          # heads per core = 8
J = HPC * D_K             # per-core projection width = 512
G = J // 128              # j-tiles per core = 4
DT = D_M // 128           # d (contraction) tiles = 8
CHUNK = 512               # tq chunk (free dim of most matmuls)
NCHUNK = T // CHUNK       # 4
NBLK = T // 128           # tk blocks = 16
F32 = mybir.dt.float32
BF16 = mybir.dt.bfloat16
NEG = -1.0e30
LAG = 3                   # attn@V lags scores by LAG blocks in the stream

_cached_nc = {}


def build_nc(loop_n=None):
    """loop_n: if set, wrap the whole compute body (projections + attention
    + stores, including x-chunk DMA loads) in a HW For_i loop for loop-slope
    timing.  Weights/constants load once outside the loop; each iteration
    recomputes the same (correct) output."""
    nc = bacc.Bacc(None)

    xq = nc.declare_dram_parameter("xq_t", [D_M, T], BF16, isOutput=False)
    xk = nc.declare_dram_parameter("xk_t", [D_M, T], BF16, isOutput=False)
    xv = nc.declare_dram_parameter("xv_t", [D_M, T], BF16, isOutput=False)
    wq = nc.declare_dram_parameter("wq", [D_M, J], BF16, isOutput=False)
    wk = nc.declare_dram_parameter("wk", [D_M, J], BF16, isOutput=False)
    wv = nc.declare_dram_parameter("wv", [D_M, J], BF16, isOutput=False)
    bq = nc.declare_dram_parameter("bq2", [128, G], F32, isOutput=False)
    bk = nc.declare_dram_parameter("bk2", [128, G], F32, isOutput=False)
    bvb = nc.declare_dram_parameter("bvb", [128, J], F32, isOutput=False)
    wo = nc.declare_dram_parameter("wo", [J, D_M], BF16, isOutput=False)
    bob = nc.declare_dram_parameter("bob", [128, D_M], F32, isOutput=False)
    padb = nc.declare_dram_parameter("padb", [128, NBLK], F32, isOutput=False)
    out = nc.declare_dram_parameter("out", [T, D_M], BF16, isOutput=True)

    Exp = mybir.ActivationFunctionType.Exp
    Ident = mybir.ActivationFunctionType.Identity
    GE = mybir.AluOpType.is_ge

    with tile.TileContext(nc) as tc:
        with (
            tc.tile_pool(name="consts", bufs=1) as cpool,
            tc.tile_pool(name="wproj", bufs=1) as wpool,
            tc.tile_pool(name="persist", bufs=1) as ppool,
            tc.tile_pool(name="xs", bufs=24) as xpool,
            tc.tile_pool(name="qht", bufs=5) as qpool,
            tc.tile_pool(name="ant", bufs=8) as apool,
            tc.tile_pool(name="exps", bufs=5) as epool,
            tc.tile_pool(name="rec", bufs=2) as rpool,
            tc.tile_pool(name="osb", bufs=3) as opool,
            tc.tile_pool(name="ps_x", bufs=2, space="PSUM") as psum_px,
            tc.tile_pool(name="ps_a", bufs=4, space="PSUM") as psum_ap,
        ):
            # ---- weight/constant DMAs, first-needed first ----------------
            # DMA queues drain in emission order, so later-needed weights are
            # emitted at the program point just before their consumer phase
            # (via the load hooks below); in loop-timing mode all weight
            # loads happen once, before the loop.
            wk_t, xk_c0_pre = [], []
            for d in range(DT):
                t_ = wpool.tile([128, J], BF16, name=f"wk_t{d}", tag=f"wk{d}")
                nc.sync.dma_start(out=t_[:, :], in_=wk[d * 128:(d + 1) * 128, :])
                wk_t.append(t_)
                if loop_n is None:
                    # Interleave the first xk chunk with wk so the first
                    # K-projection chain starts after ~2 tiles of DMA.
                    x_ = xpool.tile([128, CHUNK], BF16, name=f"xk0_{d}", tag="xs")
                    nc.sync.dma_start(out=x_[:, :], in_=xk[d * 128:(d + 1) * 128, 0:CHUNK])
                    xk_c0_pre.append(x_)
            bk_t = cpool.tile([128, G], F32, name="bk_t", tag="bk")
            nc.sync.dma_start(out=bk_t[:, :], in_=bk[:, :])
            bq_t = cpool.tile([128, G], F32, name="bq_t", tag="bq")
            nc.sync.dma_start(out=bq_t[:, :], in_=bq[:, :])
            padb_t = cpool.tile([128, NBLK], F32, name="padb_t", tag="padb")
            nc.sync.dma_start(out=padb_t[:, :], in_=padb[:, :])

            wv_t, wq_t, wo_t = [], [], []
            bvb_t, bob_t = [None], [None]

            def load_wv():
                for d in range(DT):
                    t_ = wpool.tile([128, J], BF16, name=f"wv_t{d}", tag=f"wv{d}")
                    nc.sync.dma_start(out=t_[:, :], in_=wv[d * 128:(d + 1) * 128, :])
                    wv_t.append(t_)
                bvb_t[0] = cpool.tile([128, J], F32, name="bvb_t", tag="bvb")
                nc.sync.dma_start(out=bvb_t[0][:, :], in_=bvb[:, :])

            def load_wq_wo():
                for d in range(DT):
                    t_ = wpool.tile([128, J], BF16, name=f"wq_t{d}", tag=f"wq{d}")
                    nc.sync.dma_start(out=t_[:, :], in_=wq[d * 128:(d + 1) * 128, :])
                    wq_t.append(t_)
                for g in range(G):
                    t_ = wpool.tile([128, D_M], BF16, name=f"wo_t{g}", tag=f"wo{g}")
                    nc.sync.dma_start(out=t_[:, :], in_=wo[g * 128:(g + 1) * 128, :])
                    wo_t.append(t_)
                bob_t[0] = cpool.tile([128, D_M], F32, name="bob_t", tag="bob")
                nc.sync.dma_start(out=bob_t[0][:, :], in_=bob[:, :])

            # Persistent activations.
            khT = []  # khT[g]: [128, T] — heads 2g (rows 0-63), 2g+1 (64-127)
            for g in range(G):
                khT.append(ppool.tile([128, T], BF16, name=f"khT{g}", tag=f"khT{g}"))
            vh = []   # vh[i]: [128, 520] — per head h: cols 65h..65h+63 = v, 65h+64 = 1
            for i in range(NBLK):
                vh.append(ppool.tile([128, 520], BF16, name=f"vh{i}", tag=f"vh{i}"))

            def load_x_chunk(xdram, c, label):
                tiles = []
                for d in range(DT):
                    t_ = xpool.tile([128, CHUNK], BF16, name=f"{label}{c}_{d}", tag="xs")
                    nc.sync.dma_start(
                        out=t_[:, :],
                        in_=xdram[d * 128:(d + 1) * 128, c * CHUNK:(c + 1) * CHUNK],
                    )
                    tiles.append(t_)
                return tiles

            def proj_psum(idx, name):
                """Alternate projection-chain PSUM between the px ring (2
                slots) and the pa ring (4 slots, idle outside attention) so
                back-to-back chains never wait on a draining slot."""
                if idx % 2 == 0:
                    return psum_px.tile([128, CHUNK], F32, name=name,
                                        tag="px", padded_shape=[128, 2 * CHUNK])
                return psum_ap.tile([128, CHUNK], F32, name=name, tag="pa")

            def body(load_hooks=True):
                # ---- K projection ----------------------------------------
                for c in range(NCHUNK):
                    if c == 0 and xk_c0_pre:
                        xk_c = xk_c0_pre
                    else:
                        xk_c = load_x_chunk(xk, c, "xk")
                    for g in range(G):
                        ps = proj_psum(g, f"psK{c}_{g}")
                        for d in range(DT):
                            nc.tensor.matmul(
                                ps[:, :], wk_t[d][:, g * 128:(g + 1) * 128],
                                xk_c[d][:, :],
                                start=(d == 0), stop=(d == DT - 1),
                            )
                        # drains alternate DVE/ACT so neither engine's
                        # PSUM-read latency gates the slot recycle
                        if g % 2 == 0:
                            nc.vector.tensor_scalar_add(
                                khT[g][:, c * CHUNK:(c + 1) * CHUNK], ps[:, :],
                                bk_t[:, g:g + 1],
                            )
                        else:
                            nc.scalar.activation(
                                khT[g][:, c * CHUNK:(c + 1) * CHUNK], ps[:, :],
                                Ident, bias=bk_t[:, g:g + 1],
                            )

                # ---- V projection ----------------------------------------
                if load_hooks:
                    load_wv()
                for c in range(NCHUNK):
                    xv_c = load_x_chunk(xv, c, "xv")
                    for tl in range(4):
                        i = 4 * c + tl
                        ps = proj_psum(tl, f"psV{i}")
                        for d in range(DT):
                            nc.tensor.matmul(
                                ps[:, :],
                                xv_c[d][:, tl * 128:(tl + 1) * 128],
                                wv_t[d][:, :],
                                start=(d == 0), stop=(d == DT - 1),
                            )
                        dst = vh[i][:, 0:J + HPC].rearrange("p (h e) -> p h e", e=65)
                        nc.vector.tensor_add(
                            dst[:, :, 0:64],
                            ps[:, :].rearrange("p (h e) -> p h e", e=64),
                            bvb_t[0][:, :].rearrange("p (h e) -> p h e", e=64),
                        )
                        nc.vector.memset(dst[:, :, 64:65], 1.0)
                if load_hooks:
                    load_wq_wo()

                # ---- per-chunk pipelined attention -----------------------
                def emit_qproj(c, g, xq_c):
                    qt = qpool.tile([128, CHUNK], BF16, name=f"qht{c}_{g}", tag="qht")
                    ps = psum_px.tile([128, CHUNK], F32, name=f"psQ{c}_{g}",
                                      tag="px", padded_shape=[128, 2 * CHUNK])
                    for d in range(DT):
                        nc.tensor.matmul(
                            ps[:, :], wq_t[d][:, g * 128:(g + 1) * 128],
                            xq_c[d][:, :],
                            start=(d == 0), stop=(d == DT - 1),
                        )
                    nc.vector.tensor_scalar_add(qt[:, :], ps[:, :], bq_t[:, g:g + 1])
                    return qt

                def emit_psO(cprev, mt, e, ant_list, alt=None):
                    row0 = (4 * cprev + mt) * 128
                    if alt is None:
                        ps = psum_px.tile([128, CHUNK], F32,
                                          name=f"psO{cprev}_{mt}_{e}",
                                          tag="px", padded_shape=[128, 2 * CHUNK])
                    else:
                        ps = proj_psum(alt, f"psO{cprev}_{mt}_{e}")
                    for g in range(G):
                        nc.tensor.matmul(
                            ps[:, :],
                            ant_list[g][:, mt * 128:(mt + 1) * 128],
                            wo_t[g][:, e * CHUNK:(e + 1) * CHUNK],
                            start=(g == 0), stop=(g == G - 1),
                        )
                    ob = opool.tile([128, CHUNK], BF16, name=f"ob{cprev}_{mt}_{e}", tag="ob")
                    nc.vector.tensor_add(ob[:, :], ps[:, :],
                                         bob_t[0][:, e * CHUNK:(e + 1) * CHUNK])
                    nc.sync.dma_start(
                        out=out[row0:row0 + 128, e * CHUNK:(e + 1) * CHUNK],
                        in_=ob[:, :],
                    )

                ant_prev = None
                xq_c = load_x_chunk(xq, 0, "xq")
                qht0_next = emit_qproj(0, 0, xq_c)
                for c in range(NCHUNK):
                    nb = 4 * c + 4  # causal: tk blocks 0..nb-1
                    qht = [None] * G
                    qht[0] = qht0_next
                    ant = [apool.tile([128, CHUNK], BF16, name=f"ant{c}_{g}", tag="ant")
                           for g in range(G)]
                    psa = [None] * G
                    blocks = [(g, bk) for g in range(G) for bk in range(nb)]
                    Lb = len(blocks)

                    # Filler chains interleaved into the stream: Q projections
                    # for groups 1-3 of this chunk (just-in-time, each well
                    # before its group starts), the output projection of the
                    # previous chunk (spread evenly), and the xq prefetch + g0
                    # Q projection of the NEXT chunk (late, hiding the
                    # chunk-boundary drain).
                    fillers = {}

                    def place(pos, item):
                        pos = max(0, min(Lb - 1, pos))
                        for cand in list(range(pos, Lb)) + list(range(pos - 1, -1, -1)):
                            if cand not in fillers:
                                fillers[cand] = item
                                return
                        raise AssertionError("no filler slot free")
                    for g in range(1, G):
                        place((g - 1) * nb + 1, ("psQ", g))
                    if ant_prev is not None:
                        for k in range(8):
                            place(3 + (k * Lb) // 8, ("psO", k // 2, k % 2))
                    if c + 1 < NCHUNK:
                        place(Lb // 2, ("xq_next",))
                        place(Lb - 4, ("pq_next",))

                    es_ring = {}

                    def emit_norm(c, g):
                        for hh in range(2):
                            h = 2 * g + hh
                            # NB: partition_broadcast reads partition 0 of the
                            # underlying tile regardless of the AP's partition
                            # offset, so the reciprocal must land on partition 0.
                            rc = rpool.tile([128, CHUNK], F32, name=f"rc{c}_{h}", tag="rc")
                            nc.vector.reciprocal(rc[0:1, :], psa[g][hh][64:65, :])
                            rb = rpool.tile([128, CHUNK], F32, name=f"rb{c}_{h}", tag="rb")
                            nc.gpsimd.partition_broadcast(rb[0:64, :], rc[0:1, :])
                            nc.vector.tensor_mul(
                                ant[g][hh * 64:(hh + 1) * 64, :],
                                psa[g][hh][0:64, :], rb[0:64, :],
                            )

                    def emit_A(i):
                        g, bk = blocks[i]
                        m = bk - 4 * c
                        cs = 128 * m if m >= 0 else 0
                        est = es_ring.pop(i)
                        if psa[g] is None:
                            psa[g] = [psum_ap.tile([65, CHUNK], F32,
                                                   name=f"psA{c}_{2 * g + hh}", tag="pa")
                                      for hh in range(2)]
                        first = (bk == 0)
                        last = (bk == nb - 1)
                        for hh in range(2):
                            h = 2 * g + hh
                            nc.tensor.matmul(
                                psa[g][hh][:, cs:],
                                vh[bk][:, 65 * h:65 * h + 65],
                                est[:, 512 * hh + cs:512 * hh + 512],
                                start=first, stop=last,
                            )
                        if last:
                            emit_norm(c, g)

                    for i, (g, bk) in enumerate(blocks):
                        m = bk - 4 * c
                        cs = 128 * m if m >= 0 else 0
                        ps_s = psum_px.tile([128, 2 * CHUNK], F32,
                                            name=f"psS{c}_{g}_{bk}", tag="px")
                        for hh in range(2):
                            nc.tensor.matmul(
                                ps_s[:, 512 * hh + cs:512 * hh + 512],
                                khT[g][hh * 64:(hh + 1) * 64, bk * 128:(bk + 1) * 128],
                                qht[g][hh * 64:(hh + 1) * 64, cs:],
                                start=True, stop=True,
                            )
                        es = epool.tile([128, 2 * CHUNK], BF16,
                                        name=f"es{c}_{g}_{bk}", tag="es")
                        if cs > 0:
                            nc.scalar.activation(
                                es[:, :].rearrange("p (h e) -> p h e", h=2)[:, :, cs:],
                                ps_s[:, :].rearrange("p (h e) -> p h e", h=2)[:, :, cs:],
                                Exp, bias=padb_t[:, bk:bk + 1], scale=0.125,
                            )
                        else:
                            nc.scalar.activation(
                                es[:, :], ps_s[:, :], Exp,
                                bias=padb_t[:, bk:bk + 1], scale=0.125,
                            )
                        if m >= 0:
                            # Zero the strictly-upper triangle of the diagonal
                            # 128-col window on both heads: keep where
                            # col - partition >= 0.
                            sel = es[:, :].rearrange("p (h e) -> p h e", h=2)[:, :, cs:cs + 128]
                            nc.gpsimd.affine_select(
                                sel, sel, pattern=[[0, 2], [1, 128]],
                                compare_op=GE, fill=0.0,
                                base=0, channel_multiplier=-1,
                            )
                        es_ring[i] = es
                        if i >= LAG:
                            emit_A(i - LAG)
                        f = fillers.get(i)
                        if f is not None:
                            if f[0] == "psQ":
                                qht[f[1]] = emit_qproj(c, f[1], xq_c)
                            elif f[0] == "psO":
                                emit_psO(c - 1, f[1], f[2], ant_prev)
                            elif f[0] == "xq_next":
                                xq_next = load_x_chunk(xq, c + 1, "xq")
                            else:  # pq_next
                                qht0_next = emit_qproj(c + 1, 0, xq_next)
                    for i in range(max(0, Lb - LAG), Lb):
                        emit_A(i)
                    if c + 1 < NCHUNK:
                        xq_c = xq_next
                    ant_prev = ant

                # ---- drain: output projection of the last chunk ----------
                for mt in range(4):
                    for e in range(2):
                        emit_psO(NCHUNK - 1, mt, e, ant_prev, alt=2 * mt + e)

            if loop_n is not None:
                load_wv()
                load_wq_wo()
                with tc.For_i(0, loop_n, 1):
                    body(load_hooks=False)
            else:
                body()

    nc.finalize()
    return nc


def get_nc(loop_n=None):
    key = loop_n
    if key not in _cached_nc:
        _cached_nc[key] = build_nc(loop_n)
    return _cached_nc[key]


def make_in_maps(q, k, v, pad_mask, Wq, bq, Wk, bk, Wv, bv, Wo, bo):
    """Host-side sharding: core c -> batch c//2, head-group c%2."""
    f = np.float32
    bf = ml_dtypes.bfloat16
    in_maps = []
    xT = {}
    for n in range(N_B):
        xT[n] = (
            np.ascontiguousarray(np.asarray(q[n], f).T.astype(bf)),
            np.ascontiguousarray(np.asarray(k[n], f).T.astype(bf)),
            np.ascontiguousarray(np.asarray(v[n], f).T.astype(bf)),
        )
    for c in range(N_CORES):
        n, grp = divmod(c, 2)
        js = slice(grp * J, (grp + 1) * J)
        pb = np.where(np.asarray(pad_mask[n]) == 0, NEG, 0.0).astype(f)
        in_maps.append({
            "xq_t": xT[n][0], "xk_t": xT[n][1], "xv_t": xT[n][2],
            "wq": np.ascontiguousarray(np.asarray(Wq, f)[:, js].astype(bf)),
            "wk": np.ascontiguousarray(np.asarray(Wk, f)[:, js].astype(bf)),
            "wv": np.ascontiguousarray(np.asarray(Wv, f)[:, js].astype(bf)),
            "bq2": np.ascontiguousarray(np.asarray(bq, f)[js].reshape(G, 128).T),
            "bk2": np.ascontiguousarray(np.asarray(bk, f)[js].reshape(G, 128).T),
            "bvb": np.broadcast_to(np.asarray(bv, f)[js], (128, J)).copy(),
            "wo": np.ascontiguousarray(np.asarray(Wo, f)[js, :].astype(bf)),
            "bob": (np.broadcast_to(np.asarray(bo, f), (128, D_M)).copy()
                    if grp == 0 else np.zeros((128, D_M), f)),
            "padb": np.ascontiguousarray(pb.reshape(NBLK, 128).T),
        })
    return in_maps


def kernel(**inputs) -> np.ndarray:
    nc = get_nc()
    in_maps = make_in_maps(**inputs)
    res = run_bass_kernel_spmd(nc, in_maps, list(range(N_CORES))).results
    out = np.empty((N_B, T, D_M), np.float32)
    for n in range(N_B):
        out[n] = (np.asarray(res[2 * n]["out"], np.float32)
                  + np.asarray(res[2 * n + 1]["out"], np.float32))
    return out
